# revision 14
# baseline (speedup 1.0000x reference)
"""Trainium2 Bass kernel for the BitwiseAutoencoder problem.

Pipeline (per core, data-parallel over batch: 8 of 64 batches per core):
  1. conv1d(1->256, k=256, stride=16, pad=256) as fp32r matmuls (1 cycle/row
     for free-dim >= 256) against a stride-replicated frame matrix R.
  2. relu + bias fused into PSUM eviction (split across Act/DVE engines),
     output H kept in SBUF as bf16; batchnorm statistics via bn_stats/bn_aggr,
     all-gathered across the 8 cores.
  3. BN affine folded into the transposed-conv weights (a*W2, bf16) and a
     per-phase bias vector (from d = beta - a*mu).
  4. convT(256->1, k=256, stride=16) as bf16 matmuls producing per-tap
     partials; the 16->1 tap-group fold is realized with a tap-half rhs shift
     inside PSUM plus an 8-way DMA regroup (one DMA per tap-group covering
     all 8 batches) and a vector-add tree.

Output is written in a transposed [b, phase, frame] DRAM layout (fully
contiguous DMA); the host reassembles the [B, 1, T] result.

The kernel is self-contained: shapes/sharding are hardcoded for
x: [64, 1, 32768] f32 and 8 NeuronCores.
"""

import numpy as np

import concourse.bass as bass
from concourse import bacc, mybir, tile
from concourse.bass_utils import run_bass_kernel_spmd

N_CORES = 8
B_FULL = 64
BPC = B_FULL // N_CORES  # 8 batches per core
T = 32768
K = 256
S = 16
BN_EPS = 1e-5

XP = T + 2 * K  # padded x length per batch (33280)
L = (T + 2 * K - K) // S + 1  # conv output length (2065)
RW = 2074  # R width: l+g in [0, 2065+8]
PW = XP // S  # 2080 phase columns
PWD = PW + 2  # padded phase columns in DRAM (R reads 2 cols past PW)
WOUT = 2048  # output frames per batch (w in [16, 2064))

# conv matmul tiles (fp32r requires EVEN free-dim counts; cover 2066 columns,
# the last one garbage) and bn_stats tiles (EQUAL-WIDTH groups so bn_aggr
# weights them equally; exactly the 2065 real columns)
LPAD = 2066
MM_TILES = [(0, 414), (414, 414), (828, 414), (1242, 414), (1656, 410)]
BN_TILES = [(413 * i, 413) for i in range(5)]

# deconv output tiles over w in [16, 2064); smallest tile last so the
# non-overlappable scatter+fold+store tail is short
U_TILES = [(16, 640), (656, 640), (1296, 512), (1808, 256)]

F32 = mybir.dt.float32
F32R = mybir.dt.float32r
BF16 = mybir.dt.bfloat16
AF = mybir.ActivationFunctionType


def _build():
    nc = bacc.Bacc("TRN2", target_bir_lowering=False, debug=False)

    # ---- external I/O ----
    # x in phase layout: x_ph[b, p, n] = x_pad[b, 16n + p]; fp32r (matmul
    # consumes it directly at 1 cycle/row)
    xph_t = nc.dram_tensor("x_ph", [BPC, 16, PWD], F32R, kind="ExternalInput")
    w1t_t = nc.dram_tensor("w1t", [K, K], F32R, kind="ExternalInput")
    bias1_t = nc.dram_tensor("bias1", [K], F32, kind="ExternalInput")
    w2_t = nc.dram_tensor("w2", [K, K], F32, kind="ExternalInput")  # [ch k, tap j]
    w2fold_t = nc.dram_tensor("w2fold", [K, 16], F32, kind="ExternalInput")
    gamma_t = nc.dram_tensor("gamma", [K], F32, kind="ExternalInput")
    beta_t = nc.dram_tensor("beta", [K], F32, kind="ExternalInput")
    cb16_t = nc.dram_tensor("cb16", [16], F32, kind="ExternalInput")
    # transposed output layout: y_ph[b, p, w] = y_b[16*w + p]
    y_t = nc.dram_tensor("y", [BPC, 16, WOUT], F32, kind="ExternalOutput")

    with tile.TileContext(nc) as tc:
        with (
            tc.tile_pool(name="persist", bufs=1) as persist,
            tc.tile_pool(name="rpool", bufs=2) as rpool,
            tc.tile_pool(name="of2pool", bufs=2) as of2pool,
            tc.tile_pool(name="t4pool", bufs=2) as t4pool,
            tc.tile_pool(name="yacc", bufs=2) as yaccpool,
            tc.tile_pool(name="smalls", bufs=1) as smalls,
            tc.tile_pool(name="psum_conv", bufs=2, space="PSUM") as psum_conv,
            tc.tile_pool(name="psum_dec", bufs=5, space="PSUM") as psum_dec,
            tc.tile_pool(name="psum_cp", bufs=1, space="PSUM") as psum_cp,
            tc.tile_pool(name="dram", bufs=1, space="DRAM") as dram,
        ):
            # ---- load weights/constants into SBUF ----
            w1t_sb = []
            for h in range(2):
                wh = persist.tile([128, K], F32R, tag=f"w1t{h}", name=f"w1t{h}")
                nc.scalar.dma_start(out=wh[:], in_=w1t_t[128 * h:128 * (h + 1), :])
                w1t_sb.append(wh)
            w2_sb = []  # per ch-half kc: [128, 256] (rows: ch k-128kc, cols: tap j)
            w2fold_sb = []
            for kc in range(2):
                wt = persist.tile([128, K], F32, tag=f"w2{kc}", name=f"w2{kc}")
                nc.scalar.dma_start(out=wt[:], in_=w2_t[128 * kc:128 * (kc + 1), :])
                w2_sb.append(wt)
                wf = persist.tile([128, 16], F32, tag=f"w2fold{kc}", name=f"w2fold{kc}")
                nc.scalar.dma_start(out=wf[:], in_=w2fold_t[128 * kc:128 * (kc + 1), :])
                w2fold_sb.append(wf)
            bias1_sb, gamma_sb, beta_sb = [], [], []
            for cc in range(2):
                for lst, src, nm in ((bias1_sb, bias1_t, "b1"), (gamma_sb, gamma_t, "gm"),
                                     (beta_sb, beta_t, "bt")):
                    tl = persist.tile([128, 1], F32, tag=f"v{cc}_{nm}", name=f"vec{cc}_{nm}")
                    nc.scalar.dma_start(out=tl[:], in_=src[128 * cc:128 * (cc + 1)])
                    lst.append(tl)
            cb_sb = persist.tile([16, 1], F32, tag="cb")
            nc.scalar.dma_start(out=cb_sb[:], in_=cb16_t[:])
            eps_sb = persist.tile([128, 1], F32, tag="eps")
            nc.vector.memset(eps_sb[:], BN_EPS)

            # H: conv output (post-relu), kept in SBUF as bf16
            H = [persist.tile([128, BPC, LPAD], BF16, tag=f"H{cc}", name=f"H{cc}") for cc in range(2)]
            # bn_stats accumulator: per cc: 8 batches x 5 equal groups
            stats = [persist.tile([128, 5 * BPC, 6], F32, tag=f"st{cc}", name=f"st{cc}") for cc in range(2)]

            # ================= phase 1: conv + stats =================
            for b in range(BPC):
                # R[16g+p, l] = x_pad[16(l+g) + p] = x_ph[b, p, l+g]
                R = rpool.tile([128, RW], F32R, tag="R", name=f"R{b}")
                RH = RW // 2
                for eng, c0, cw in ((nc.sync, 0, RH), (nc.scalar, RH, RW - RH)):
                    eng.dma_start(
                        out=R[:, c0:c0 + cw],
                        in_=bass.AP(tensor=xph_t, offset=b * 16 * PWD + c0,
                                    ap=[[1, 8], [PWD, 16], [1, cw]]),
                    )
                for cc in range(2):
                    cs = slice(128 * cc, 128 * (cc + 1))
                    for gi, (l0, w) in enumerate(MM_TILES):
                        ps = psum_conv.tile([128, 416], F32, tag="pconv")
                        for h in range(2):
                            nc.tensor.matmul(
                                ps[:, :w], w1t_sb[h][:, cs],
                                R[:, l0 + 8 * h:l0 + 8 * h + w],
                                start=(h == 0), stop=(h == 1),
                            )
                        # h = relu(psum + bias); conv_scale folded into W on host
                        hdst = H[cc][:, b, l0:l0 + w]
                        if gi < 4 or cc == 0:
                            nc.scalar.activation(
                                out=hdst, in_=ps[:, :w], func=AF.Relu,
                                bias=bias1_sb[cc][:, 0:1], scale=1.0,
                            )
                        else:
                            nc.vector.tensor_scalar(
                                out=hdst, in0=ps[:, :w],
                                scalar1=bias1_sb[cc][:, 0:1], scalar2=0.0,
                                op0=mybir.AluOpType.add, op1=mybir.AluOpType.max,
                            )
                    for gi, (l0, w) in enumerate(BN_TILES):
                        nc.vector.bn_stats(
                            out=stats[cc][:, 5 * b + gi, :],
                            in_=H[cc][:, b, l0:l0 + w],
                        )

            # ================= phase 2: global BN stats =================
            bounce_in = dram.tile([2, 128, 2], F32)
            bounce_out = dram.tile([N_CORES, 2, 128, 2], F32)
            for cc in range(2):
                mv = smalls.tile([128, 2], F32, tag=f"mv{cc}", name=f"mv{cc}")
                nc.vector.bn_aggr(out=mv[:], in_=stats[cc][:])
                # pack [mean, E[h^2]] = [mean, var + mean^2]
                pk = smalls.tile([128, 2], F32, tag=f"pk{cc}", name=f"pk{cc}")
                nc.vector.tensor_mul(pk[:, 0:1], mv[:, 0:1], mv[:, 0:1])
                nc.vector.tensor_add(pk[:, 1:2], mv[:, 1:2], pk[:, 0:1])
                nc.vector.tensor_copy(pk[:, 0:1], mv[:, 0:1])
                nc.sync.dma_start(out=bounce_in[cc, :, :], in_=pk[:])
            # AllGather (cheaper than AllReduce) + local sum over cores
            nc.gpsimd.collective_compute(
                "AllGather",
                mybir.AluOpType.bypass,
                replica_groups=[list(range(N_CORES))],
                ins=[bounce_in.opt()],
                outs=[bounce_out.opt()],
            )
            a_sb, d_sb = [], []
            for cc in range(2):
                # gathered[core, cc, p, v] -> sbuf [128, 2, 8] (v, core)
                gall = smalls.tile([128, 2, N_CORES], F32, tag=f"gall{cc}", name=f"gall{cc}")
                nc.sync.dma_start(
                    out=gall[:],
                    in_=bass.AP(tensor=bounce_out.tensor,
                                offset=bounce_out.offset + cc * 256,
                                ap=[[2, 128], [1, 2], [512, N_CORES]]),
                )
                gst = smalls.tile([128, 2], F32, tag=f"gst{cc}", name=f"gst{cc}")
                nc.vector.reduce_sum(gst[:], gall[:], axis=mybir.AxisListType.X)
                # gmean = sum/8 ; gE2 = sum/8 ; gvar = gE2 - gmean^2
                gm = smalls.tile([128, 2], F32, tag=f"gm{cc}", name=f"gm{cc}")
                nc.vector.tensor_scalar_mul(gm[:], gst[:], 1.0 / N_CORES)
                gvar = smalls.tile([128, 1], F32, tag=f"gvar{cc}", name=f"gvar{cc}")
                nc.vector.tensor_mul(gvar[:], gm[:, 0:1], gm[:, 0:1])
                nc.vector.tensor_sub(gvar[:], gm[:, 1:2], gvar[:])
                sd = smalls.tile([128, 1], F32, tag=f"sd{cc}", name=f"sd{cc}")
                nc.scalar.activation(out=sd[:], in_=gvar[:], func=AF.Sqrt,
                                     bias=eps_sb[:, 0:1], scale=1.0)
                rinv = smalls.tile([128, 1], F32, tag=f"rinv{cc}", name=f"rinv{cc}")
                nc.vector.reciprocal(rinv[:], sd[:])
                a = smalls.tile([128, 1], F32, tag=f"a{cc}", name=f"a{cc}")
                nc.vector.tensor_mul(a[:], rinv[:], gamma_sb[cc][:])
                # d = beta - a * gmean
                d = smalls.tile([128, 1], F32, tag=f"d{cc}", name=f"d{cc}")
                nc.vector.tensor_mul(d[:], a[:], gm[:, 0:1])
                nc.vector.tensor_sub(d[:], beta_sb[cc][:], d[:])
                a_sb.append(a)
                d_sb.append(d)
            # fold BN scale into deconv weights (in place), then round to bf16
            w2a_bf = []
            for kc in range(2):
                nc.vector.tensor_scalar_mul(w2_sb[kc][:], w2_sb[kc][:], a_sb[kc][:, 0:1])
                wb = persist.tile([128, K], BF16, tag=f"w2a{kc}", name=f"w2a{kc}")
                nc.vector.tensor_copy(wb[:], w2_sb[kc][:])
                w2a_bf.append(wb)
            # per-phase bias: CP[p] = sum_k w2fold[k, p] d[k] + ct_scale*ct_b
            pcp = psum_cp.tile([16, 1], F32, tag="pcp")
            nc.tensor.matmul(pcp[:], w2fold_sb[0][:], d_sb[0][:], start=True, stop=False)
            nc.tensor.matmul(pcp[:], w2fold_sb[1][:], d_sb[1][:], start=False, stop=True)
            cp16 = smalls.tile([16, 1], F32, tag="cp16")
            nc.vector.tensor_add(cp16[:], pcp[:], cb_sb[:])
            cp_dram = dram.tile([16], F32)
            nc.sync.dma_start(out=cp_dram[:], in_=cp16[:])
            # (p, b)-ordered broadcast: cpb[8p + b] = cp[p]
            cpb = smalls.tile([128, 1], F32, tag="cpb")
            nc.sync.dma_start(
                out=cpb[:],
                in_=bass.AP(tensor=cp_dram.tensor, offset=cp_dram.offset,
                            ap=[[1, 16], [0, 8], [0, 1]]),
            )

            # ================= phase 3: deconv =================
            # of2[16g+p, b, n] = sum over both tap-halves (th fold via rhs
            # shift inside PSUM accumulation); t4 partitions are (p, b)
            # ordered: t4[8p + b, g, w]; y[b, 16w+p] = sum_g t4[8p+b, g, w].
            for (w0, wt) in U_TILES:
                wov = wt + 7
                of2 = of2pool.tile([128, BPC, 690], BF16, tag="of2", name=f"of2_{w0}")
                for b in range(BPC):
                    for s0 in ((0, 512) if wov > 512 else (0,)):
                        sw = min(512, wov - s0)
                        ps = psum_dec.tile([128, 512], F32, tag="pdec")
                        nmm = 0
                        for th, off in ((0, 7), (128, 15)):
                            for kc in range(2):
                                nc.tensor.matmul(
                                    ps[:, :sw], w2a_bf[kc][:, th:th + 128],
                                    H[kc][:, b, w0 - off + s0:w0 - off + s0 + sw],
                                    start=(nmm == 0), stop=(nmm == 3),
                                )
                                nmm += 1
                        if sw > 256:
                            nc.scalar.activation(
                                out=of2[:, b, s0:s0 + sw], in_=ps[:, :sw],
                                func=AF.Copy,
                            )
                        else:
                            nc.vector.tensor_copy(of2[:, b, s0:s0 + sw], ps[:, :sw])
                # regroup: one DMA per tap-group m covers all 8 batches;
                # t4[8p + b, m, w] = of2[16m + p, b, w + 7 - m]
                t4 = t4pool.tile([128, 8, 640], BF16, tag="t4", name=f"t4_{w0}")
                for m in range(8):
                    eng = nc.gpsimd if m % 2 == 0 else nc.sync
                    eng.dma_start(
                        out=t4[:, m, :wt],
                        in_=of2[16 * m:16 * (m + 1), :, 7 - m:7 - m + wt],
                    )
                # fold the 8 tap-groups + per-phase bias (f32 accumulation:
                # a bf16 accumulator chain costs ~8e-3 relative error)
                ac0 = yaccpool.tile([128, 640], F32, tag="ac0", name=f"ac0_{w0}")
                ac1 = yaccpool.tile([128, 640], F32, tag="ac1", name=f"ac1_{w0}")
                ya = yaccpool.tile([128, 640], F32, tag="ya", name=f"ya_{w0}")
                nc.vector.tensor_add(ac0[:, :wt], t4[:, 0, :wt], t4[:, 1, :wt])
                nc.vector.tensor_add(ac1[:, :wt], t4[:, 2, :wt], t4[:, 3, :wt])
                for m in (4, 5):
                    nc.vector.tensor_add(ac0[:, :wt], ac0[:, :wt], t4[:, m, :wt])
                    nc.vector.tensor_add(ac1[:, :wt], ac1[:, :wt], t4[:, m + 2, :wt])
                # ya = (ac0 + cpb) + ac1
                nc.vector.scalar_tensor_tensor(
                    out=ya[:, :wt], in0=ac0[:, :wt], scalar=cpb[:, 0:1],
                    in1=ac1[:, :wt], op0=mybir.AluOpType.add,
                    op1=mybir.AluOpType.add,
                )
                nc.sync.dma_start(
                    out=bass.AP(tensor=y_t, offset=w0 - 16,
                                ap=[[WOUT, 16], [16 * WOUT, 8], [1, wt]]),
                    in_=ya[:, :wt],
                )
    nc.compile()
    return nc


_NC_CACHE = None


def _get_nc():
    global _NC_CACHE
    if _NC_CACHE is None:
        _NC_CACHE = _build()
    return _NC_CACHE


def _host_prep(inputs):
    conv_w = np.asarray(inputs["conv_w"], dtype=np.float32)
    conv_b = np.asarray(inputs["conv_b"], dtype=np.float32)
    conv_gate = np.asarray(inputs["conv_gate"], dtype=np.float32)
    conv_scale = np.asarray(inputs["conv_scale"], dtype=np.float32)
    bn_gamma = np.asarray(inputs["bn_gamma"], dtype=np.float32)
    bn_beta = np.asarray(inputs["bn_beta"], dtype=np.float32)
    ct_w = np.asarray(inputs["ct_w"], dtype=np.float32)
    ct_b = np.asarray(inputs["ct_b"], dtype=np.float32)
    ct_gate = np.asarray(inputs["ct_gate"], dtype=np.float32)
    ct_scale = np.asarray(inputs["ct_scale"], dtype=np.float32)

    W1 = conv_w[:, 0, :] * (conv_gate[:, 0, :] + 1.0) * 0.5  # [c, j]
    W1 = W1 * conv_scale[:, None]
    bias1 = conv_scale * conv_b
    w1t = np.ascontiguousarray(W1.T)  # [j, c]

    W2 = ct_w[:, 0, :] * (ct_gate[:, 0, :] + 1.0) * 0.5  # [k, j]
    W2 = W2 * float(ct_scale[0])
    w2fold = np.ascontiguousarray(W2.reshape(K, 16, 16).sum(axis=1))  # [k, p]
    cb16 = np.full(16, float(ct_scale[0]) * float(ct_b[0]), dtype=np.float32)

    return {
        "w1t": w1t,
        "bias1": bias1.astype(np.float32),
        "w2": np.ascontiguousarray(W2).astype(np.float32),
        "w2fold": w2fold.astype(np.float32),
        "gamma": bn_gamma.astype(np.float32),
        "beta": bn_beta.astype(np.float32),
        "cb16": cb16,
    }


def kernel(**inputs) -> np.ndarray:
    x = np.asarray(inputs["x"], dtype=np.float32)  # [64, 1, 32768]
    shared = _host_prep(inputs)
    nc = _get_nc()

    in_maps = []
    for c in range(N_CORES):
        shard = x[BPC * c:BPC * (c + 1), 0, :]  # [8, T]
        xpad = np.zeros((BPC, XP), dtype=np.float32)
        xpad[:, K:K + T] = shard
        # phase layout: x_ph[b, p, n] = x_pad[b, 16n + p]; 2 zero pad columns
        xph = np.zeros((BPC, 16, PWD), dtype=np.float32)
        xph[:, :, :PW] = xpad.reshape(BPC, PW, 16).transpose(0, 2, 1)
        m = dict(shared)
        m["x_ph"] = xph
        in_maps.append(m)

    res = run_bass_kernel_spmd(nc, in_maps, core_ids=list(range(N_CORES)))
    # y_ph[b, p, w] = y_b[16*w + p] -> y[b, t]
    outs = []
    for c in range(N_CORES):
        yph = res.results[c]["y"]  # [BPC, 16, WOUT]
        outs.append(np.transpose(yph, (0, 2, 1)).reshape(BPC, 1, T))
    return np.concatenate(outs, axis=0).astype(np.float32)


# revision 15
# speedup vs baseline: 1.1018x; 1.1018x over previous
"""Trainium2 Bass kernel for the BitwiseAutoencoder problem.

Pipeline (per core, data-parallel over batch: 8 of 64 batches per core):
  1. conv1d(1->256, k=256, stride=16, pad=256) as fp32r matmuls (1 cycle/row
     for free-dim >= 256) against a stride-replicated frame matrix R.
  2. relu + bias fused into PSUM eviction (split across Act/DVE engines),
     output H kept in SBUF as bf16; batchnorm statistics via bn_stats/bn_aggr,
     all-gathered across the 8 cores.
  3. BN affine folded into the transposed-conv weights (a*W2, bf16) and a
     per-phase bias vector (from d = beta - a*mu).
  4. convT(256->1, k=256, stride=16) as bf16 matmuls producing per-tap
     partials; the 16->1 tap-group fold is realized with a tap-half rhs shift
     inside PSUM plus an 8-way DMA regroup (one DMA per tap-group covering
     all 8 batches) and a vector-add tree.

Output is written in a transposed [b, phase, frame] DRAM layout (fully
contiguous DMA); the host reassembles the [B, 1, T] result.

The kernel is self-contained: shapes/sharding are hardcoded for
x: [64, 1, 32768] f32 and 8 NeuronCores.
"""

import numpy as np

import concourse.bass as bass
from concourse import bacc, mybir, tile
from concourse.bass_utils import run_bass_kernel_spmd

N_CORES = 8
B_FULL = 64
BPC = B_FULL // N_CORES  # 8 batches per core
T = 32768
K = 256
S = 16
BN_EPS = 1e-5

XP = T + 2 * K  # padded x length per batch (33280)
L = (T + 2 * K - K) // S + 1  # conv output length (2065)
RW = 2074  # R width: l+g in [0, 2065+8]
PW = XP // S  # 2080 phase columns
PWD = PW + 2  # padded phase columns in DRAM (R reads 2 cols past PW)
WOUT = 2048  # output frames per batch (w in [16, 2064))

# conv matmul tiles (fp32r requires EVEN free-dim counts; cover 2066 columns,
# the last one garbage) and bn_stats tiles (EQUAL-WIDTH groups so bn_aggr
# weights them equally; exactly the 2065 real columns)
LPAD = 2066
MM_TILES = [(0, 414), (414, 414), (828, 414), (1242, 414), (1656, 410)]
BN_TILES = [(413 * i, 413) for i in range(5)]

# deconv output tiles over w in [16, 2064); smallest tile last so the
# non-overlappable scatter+fold+store tail is short
U_TILES = [(16, 640), (656, 640), (1296, 512), (1808, 256)]

F32 = mybir.dt.float32
F32R = mybir.dt.float32r
BF16 = mybir.dt.bfloat16
AF = mybir.ActivationFunctionType


def _build():
    nc = bacc.Bacc("TRN2", target_bir_lowering=False, debug=False)

    # ---- external I/O ----
    # x in phase layout: x_ph[b, p, n] = x_pad[b, 16n + p]; fp32r (matmul
    # consumes it directly at 1 cycle/row)
    xph_t = nc.dram_tensor("x_ph", [BPC, 16, PWD], F32R, kind="ExternalInput")
    w1t_t = nc.dram_tensor("w1t", [K, K], F32R, kind="ExternalInput")
    bias1_t = nc.dram_tensor("bias1", [K], F32, kind="ExternalInput")
    w2_t = nc.dram_tensor("w2", [K, K], F32, kind="ExternalInput")  # [ch k, tap j]
    w2fold_t = nc.dram_tensor("w2fold", [K, 16], F32, kind="ExternalInput")
    gamma_t = nc.dram_tensor("gamma", [K], F32, kind="ExternalInput")
    beta_t = nc.dram_tensor("beta", [K], F32, kind="ExternalInput")
    cb16_t = nc.dram_tensor("cb16", [16], F32, kind="ExternalInput")
    # transposed output layout: y_ph[b, p, w] = y_b[16*w + p]
    y_t = nc.dram_tensor("y", [BPC, 16, WOUT], F32, kind="ExternalOutput")

    with tile.TileContext(nc) as tc:
        with (
            tc.tile_pool(name="persist", bufs=1) as persist,
            tc.tile_pool(name="rpool", bufs=2) as rpool,
            tc.tile_pool(name="of2pool", bufs=2) as of2pool,
            tc.tile_pool(name="t4pool", bufs=2) as t4pool,
            tc.tile_pool(name="yacc", bufs=2) as yaccpool,
            tc.tile_pool(name="smalls", bufs=1) as smalls,
            tc.tile_pool(name="psum_conv", bufs=3, space="PSUM") as psum_conv,
            tc.tile_pool(name="psum_dec", bufs=4, space="PSUM") as psum_dec,
            tc.tile_pool(name="psum_cp", bufs=1, space="PSUM") as psum_cp,
            tc.tile_pool(name="dram", bufs=1, space="DRAM") as dram,
        ):
            # ---- load weights/constants into SBUF ----
            w1t_sb = []
            for h in range(2):
                wh = persist.tile([128, K], F32R, tag=f"w1t{h}", name=f"w1t{h}")
                nc.scalar.dma_start(out=wh[:], in_=w1t_t[128 * h:128 * (h + 1), :])
                w1t_sb.append(wh)
            w2_sb = []  # per ch-half kc: [128, 256] (rows: ch k-128kc, cols: tap j)
            w2fold_sb = []
            for kc in range(2):
                wt = persist.tile([128, K], F32, tag=f"w2{kc}", name=f"w2{kc}")
                nc.scalar.dma_start(out=wt[:], in_=w2_t[128 * kc:128 * (kc + 1), :])
                w2_sb.append(wt)
                wf = persist.tile([128, 16], F32, tag=f"w2fold{kc}", name=f"w2fold{kc}")
                nc.scalar.dma_start(out=wf[:], in_=w2fold_t[128 * kc:128 * (kc + 1), :])
                w2fold_sb.append(wf)
            bias1_sb, gamma_sb, beta_sb = [], [], []
            for cc in range(2):
                for lst, src, nm in ((bias1_sb, bias1_t, "b1"), (gamma_sb, gamma_t, "gm"),
                                     (beta_sb, beta_t, "bt")):
                    tl = persist.tile([128, 1], F32, tag=f"v{cc}_{nm}", name=f"vec{cc}_{nm}")
                    nc.scalar.dma_start(out=tl[:], in_=src[128 * cc:128 * (cc + 1)])
                    lst.append(tl)
            cb_sb = persist.tile([16, 1], F32, tag="cb")
            nc.scalar.dma_start(out=cb_sb[:], in_=cb16_t[:])
            eps_sb = persist.tile([128, 1], F32, tag="eps")
            nc.vector.memset(eps_sb[:], BN_EPS)

            # H: conv output (post-relu), kept in SBUF as bf16
            H = [persist.tile([128, BPC, LPAD], BF16, tag=f"H{cc}", name=f"H{cc}") for cc in range(2)]
            # bn_stats accumulator: per cc: 8 batches x 5 equal groups
            stats = [persist.tile([128, 5 * BPC, 6], F32, tag=f"st{cc}", name=f"st{cc}") for cc in range(2)]

            # ================= phase 1: conv + stats =================
            for b in range(BPC):
                # R[16g+p, l] = x_pad[16(l+g) + p] = x_ph[b, p, l+g]
                R = rpool.tile([128, RW], F32R, tag="R", name=f"R{b}")
                nc.sync.dma_start(
                    out=R[:],
                    in_=bass.AP(tensor=xph_t, offset=b * 16 * PWD,
                                ap=[[1, 8], [PWD, 16], [1, RW]]),
                )
                for cc in range(2):
                    cs = slice(128 * cc, 128 * (cc + 1))
                    for gi, (l0, w) in enumerate(MM_TILES):
                        ps = psum_conv.tile([128, 416], F32, tag="pconv")
                        for h in range(2):
                            nc.tensor.matmul(
                                ps[:, :w], w1t_sb[h][:, cs],
                                R[:, l0 + 8 * h:l0 + 8 * h + w],
                                start=(h == 0), stop=(h == 1),
                            )
                        # h = relu(psum + bias); conv_scale folded into W on host
                        hdst = H[cc][:, b, l0:l0 + w]
                        if gi < 4 or cc == 0:
                            nc.scalar.activation(
                                out=hdst, in_=ps[:, :w], func=AF.Relu,
                                bias=bias1_sb[cc][:, 0:1], scale=1.0,
                            )
                        else:
                            nc.vector.tensor_scalar(
                                out=hdst, in0=ps[:, :w],
                                scalar1=bias1_sb[cc][:, 0:1], scalar2=0.0,
                                op0=mybir.AluOpType.add, op1=mybir.AluOpType.max,
                            )
                    for gi, (l0, w) in enumerate(BN_TILES):
                        nc.vector.bn_stats(
                            out=stats[cc][:, 5 * b + gi, :],
                            in_=H[cc][:, b, l0:l0 + w],
                        )

            # ================= phase 2: global BN stats =================
            bounce_in = dram.tile([2, 128, 2], F32)
            bounce_out = dram.tile([N_CORES, 2, 128, 2], F32)
            for cc in range(2):
                mv = smalls.tile([128, 2], F32, tag=f"mv{cc}", name=f"mv{cc}")
                nc.vector.bn_aggr(out=mv[:], in_=stats[cc][:])
                # pack [mean, E[h^2]] = [mean, var + mean^2]
                pk = smalls.tile([128, 2], F32, tag=f"pk{cc}", name=f"pk{cc}")
                nc.vector.tensor_mul(pk[:, 0:1], mv[:, 0:1], mv[:, 0:1])
                nc.vector.tensor_add(pk[:, 1:2], mv[:, 1:2], pk[:, 0:1])
                nc.vector.tensor_copy(pk[:, 0:1], mv[:, 0:1])
                nc.sync.dma_start(out=bounce_in[cc, :, :], in_=pk[:])
            # AllGather (cheaper than AllReduce) + local sum over cores
            nc.gpsimd.collective_compute(
                "AllGather",
                mybir.AluOpType.bypass,
                replica_groups=[list(range(N_CORES))],
                ins=[bounce_in.opt()],
                outs=[bounce_out.opt()],
            )
            a_sb, d_sb = [], []
            for cc in range(2):
                # gathered[core, cc, p, v] -> sbuf [128, 2, 8] (v, core)
                gall = smalls.tile([128, 2, N_CORES], F32, tag=f"gall{cc}", name=f"gall{cc}")
                nc.sync.dma_start(
                    out=gall[:],
                    in_=bass.AP(tensor=bounce_out.tensor,
                                offset=bounce_out.offset + cc * 256,
                                ap=[[2, 128], [1, 2], [512, N_CORES]]),
                )
                gst = smalls.tile([128, 2], F32, tag=f"gst{cc}", name=f"gst{cc}")
                nc.vector.reduce_sum(gst[:], gall[:], axis=mybir.AxisListType.X)
                # gmean = sum/8 ; gE2 = sum/8 ; gvar = gE2 - gmean^2
                gm = smalls.tile([128, 2], F32, tag=f"gm{cc}", name=f"gm{cc}")
                nc.vector.tensor_scalar_mul(gm[:], gst[:], 1.0 / N_CORES)
                gvar = smalls.tile([128, 1], F32, tag=f"gvar{cc}", name=f"gvar{cc}")
                nc.vector.tensor_mul(gvar[:], gm[:, 0:1], gm[:, 0:1])
                nc.vector.tensor_sub(gvar[:], gm[:, 1:2], gvar[:])
                sd = smalls.tile([128, 1], F32, tag=f"sd{cc}", name=f"sd{cc}")
                nc.scalar.activation(out=sd[:], in_=gvar[:], func=AF.Sqrt,
                                     bias=eps_sb[:, 0:1], scale=1.0)
                rinv = smalls.tile([128, 1], F32, tag=f"rinv{cc}", name=f"rinv{cc}")
                nc.vector.reciprocal(rinv[:], sd[:])
                a = smalls.tile([128, 1], F32, tag=f"a{cc}", name=f"a{cc}")
                nc.vector.tensor_mul(a[:], rinv[:], gamma_sb[cc][:])
                # d = beta - a * gmean
                d = smalls.tile([128, 1], F32, tag=f"d{cc}", name=f"d{cc}")
                nc.vector.tensor_mul(d[:], a[:], gm[:, 0:1])
                nc.vector.tensor_sub(d[:], beta_sb[cc][:], d[:])
                a_sb.append(a)
                d_sb.append(d)
            # fold BN scale into deconv weights (in place), then round to bf16
            w2a_bf = []
            for kc in range(2):
                nc.vector.tensor_scalar_mul(w2_sb[kc][:], w2_sb[kc][:], a_sb[kc][:, 0:1])
                wb = persist.tile([128, K], BF16, tag=f"w2a{kc}", name=f"w2a{kc}")
                nc.vector.tensor_copy(wb[:], w2_sb[kc][:])
                w2a_bf.append(wb)
            # per-phase bias: CP[p] = sum_k w2fold[k, p] d[k] + ct_scale*ct_b
            pcp = psum_cp.tile([16, 1], F32, tag="pcp")
            nc.tensor.matmul(pcp[:], w2fold_sb[0][:], d_sb[0][:], start=True, stop=False)
            nc.tensor.matmul(pcp[:], w2fold_sb[1][:], d_sb[1][:], start=False, stop=True)
            cp16 = smalls.tile([16, 1], F32, tag="cp16")
            nc.vector.tensor_add(cp16[:], pcp[:], cb_sb[:])
            cp_dram = dram.tile([16], F32)
            nc.sync.dma_start(out=cp_dram[:], in_=cp16[:])
            # (p, b)-ordered broadcast: cpb[8p + b] = cp[p]
            cpb = smalls.tile([128, 1], F32, tag="cpb")
            nc.sync.dma_start(
                out=cpb[:],
                in_=bass.AP(tensor=cp_dram.tensor, offset=cp_dram.offset,
                            ap=[[1, 16], [0, 8], [0, 1]]),
            )

            # ================= phase 3: deconv =================
            # of2[16g+p, b, n] = sum over both tap-halves (th fold via rhs
            # shift inside PSUM accumulation); t4 partitions are (p, b)
            # ordered: t4[8p + b, g, w]; y[b, 16w+p] = sum_g t4[8p+b, g, w].
            for (w0, wt) in U_TILES:
                wov = wt + 7
                of2 = of2pool.tile([128, BPC, 690], BF16, tag="of2", name=f"of2_{w0}")
                for b in range(BPC):
                    for s0 in ((0, 512) if wov > 512 else (0,)):
                        sw = min(512, wov - s0)
                        ps = psum_dec.tile([128, 512], F32, tag="pdec")
                        nmm = 0
                        for th, off in ((0, 7), (128, 15)):
                            for kc in range(2):
                                nc.tensor.matmul(
                                    ps[:, :sw], w2a_bf[kc][:, th:th + 128],
                                    H[kc][:, b, w0 - off + s0:w0 - off + s0 + sw],
                                    start=(nmm == 0), stop=(nmm == 3),
                                )
                                nmm += 1
                        if sw > 256:
                            nc.scalar.activation(
                                out=of2[:, b, s0:s0 + sw], in_=ps[:, :sw],
                                func=AF.Copy,
                            )
                        else:
                            nc.vector.tensor_copy(of2[:, b, s0:s0 + sw], ps[:, :sw])
                # regroup: one DMA per tap-group m covers all 8 batches;
                # t4[8p + b, m, w] = of2[16m + p, b, w + 7 - m]
                t4 = t4pool.tile([128, 8, 640], BF16, tag="t4", name=f"t4_{w0}")
                for m in range(8):
                    eng = nc.gpsimd if m % 2 == 0 else nc.sync
                    eng.dma_start(
                        out=t4[:, m, :wt],
                        in_=of2[16 * m:16 * (m + 1), :, 7 - m:7 - m + wt],
                    )
                # fold the 8 tap-groups + per-phase bias (f32 accumulation:
                # a bf16 accumulator chain costs ~8e-3 relative error)
                ac0 = yaccpool.tile([128, 640], F32, tag="ac0", name=f"ac0_{w0}")
                ac1 = yaccpool.tile([128, 640], F32, tag="ac1", name=f"ac1_{w0}")
                ya = yaccpool.tile([128, 640], F32, tag="ya", name=f"ya_{w0}")
                nc.vector.tensor_add(ac0[:, :wt], t4[:, 0, :wt], t4[:, 1, :wt])
                nc.vector.tensor_add(ac1[:, :wt], t4[:, 2, :wt], t4[:, 3, :wt])
                for m in (4, 5):
                    nc.vector.tensor_add(ac0[:, :wt], ac0[:, :wt], t4[:, m, :wt])
                    nc.vector.tensor_add(ac1[:, :wt], ac1[:, :wt], t4[:, m + 2, :wt])
                # ya = (ac0 + cpb) + ac1
                nc.vector.scalar_tensor_tensor(
                    out=ya[:, :wt], in0=ac0[:, :wt], scalar=cpb[:, 0:1],
                    in1=ac1[:, :wt], op0=mybir.AluOpType.add,
                    op1=mybir.AluOpType.add,
                )
                nc.sync.dma_start(
                    out=bass.AP(tensor=y_t, offset=w0 - 16,
                                ap=[[WOUT, 16], [16 * WOUT, 8], [1, wt]]),
                    in_=ya[:, :wt],
                )
    nc.compile()
    return nc


_NC_CACHE = None


def _get_nc():
    global _NC_CACHE
    if _NC_CACHE is None:
        _NC_CACHE = _build()
    return _NC_CACHE


def _host_prep(inputs):
    conv_w = np.asarray(inputs["conv_w"], dtype=np.float32)
    conv_b = np.asarray(inputs["conv_b"], dtype=np.float32)
    conv_gate = np.asarray(inputs["conv_gate"], dtype=np.float32)
    conv_scale = np.asarray(inputs["conv_scale"], dtype=np.float32)
    bn_gamma = np.asarray(inputs["bn_gamma"], dtype=np.float32)
    bn_beta = np.asarray(inputs["bn_beta"], dtype=np.float32)
    ct_w = np.asarray(inputs["ct_w"], dtype=np.float32)
    ct_b = np.asarray(inputs["ct_b"], dtype=np.float32)
    ct_gate = np.asarray(inputs["ct_gate"], dtype=np.float32)
    ct_scale = np.asarray(inputs["ct_scale"], dtype=np.float32)

    W1 = conv_w[:, 0, :] * (conv_gate[:, 0, :] + 1.0) * 0.5  # [c, j]
    W1 = W1 * conv_scale[:, None]
    bias1 = conv_scale * conv_b
    w1t = np.ascontiguousarray(W1.T)  # [j, c]

    W2 = ct_w[:, 0, :] * (ct_gate[:, 0, :] + 1.0) * 0.5  # [k, j]
    W2 = W2 * float(ct_scale[0])
    w2fold = np.ascontiguousarray(W2.reshape(K, 16, 16).sum(axis=1))  # [k, p]
    cb16 = np.full(16, float(ct_scale[0]) * float(ct_b[0]), dtype=np.float32)

    return {
        "w1t": w1t,
        "bias1": bias1.astype(np.float32),
        "w2": np.ascontiguousarray(W2).astype(np.float32),
        "w2fold": w2fold.astype(np.float32),
        "gamma": bn_gamma.astype(np.float32),
        "beta": bn_beta.astype(np.float32),
        "cb16": cb16,
    }


def kernel(**inputs) -> np.ndarray:
    x = np.asarray(inputs["x"], dtype=np.float32)  # [64, 1, 32768]
    shared = _host_prep(inputs)
    nc = _get_nc()

    in_maps = []
    for c in range(N_CORES):
        shard = x[BPC * c:BPC * (c + 1), 0, :]  # [8, T]
        xpad = np.zeros((BPC, XP), dtype=np.float32)
        xpad[:, K:K + T] = shard
        # phase layout: x_ph[b, p, n] = x_pad[b, 16n + p]; 2 zero pad columns
        xph = np.zeros((BPC, 16, PWD), dtype=np.float32)
        xph[:, :, :PW] = xpad.reshape(BPC, PW, 16).transpose(0, 2, 1)
        m = dict(shared)
        m["x_ph"] = xph
        in_maps.append(m)

    res = run_bass_kernel_spmd(nc, in_maps, core_ids=list(range(N_CORES)))
    # y_ph[b, p, w] = y_b[16*w + p] -> y[b, t]
    outs = []
    for c in range(N_CORES):
        yph = res.results[c]["y"]  # [BPC, 16, WOUT]
        outs.append(np.transpose(yph, (0, 2, 1)).reshape(BPC, 1, T))
    return np.concatenate(outs, axis=0).astype(np.float32)
